# revision 2
# baseline (speedup 1.0000x reference)
"""TRN2 Bass kernel for nn_CNNDSTv2_batch: out = mobius16(zeta16(M[:,0]) * zeta16(M[:,1])).

v4 over v3: half-bank PSUM tiles (U/V, z-halves, o-halves separate) so pools
rotate without blocking the PE; dedicated 2-bank PSUM pool for PE transposes;
input DMA prefetched one group ahead; transposes balanced across resources:
zeta-c0 on XBAR(sync), zeta-c1 on PE, mobius hi-half on XBAR(scalar) + lo-half
on PE; S1 bit-combinations via DVE presums (vector+gpsimd).
"""
import sys
import os
import functools

sys.path.insert(0, "/opt/trn_rl_repo")
import numpy as np

BATCH = 512
L = 65536
NCORES = 8
BPC = BATCH // NCORES
PAIRS = BPC // 2
OSCALE = float(2.0 ** -16)

GROUP = int(os.environ.get("GROUP", "4"))


def _pc(v):
    return bin(v).count("1")


def _constants():
    k = np.arange(128)
    sup = (k[:, None] & k[None, :]) == k[None, :]
    AT7 = sup.astype(np.float16)
    pc = np.array([_pc(i) for i in range(128)])
    sign = (-1.0) ** (pc[:, None] - pc[None, :])
    BT7 = (sup * sign).astype(np.float16)
    return AT7, BT7


def _build():
    import concourse.bacc as bacc
    import concourse.tile as tile
    import concourse.mybir as mybir

    dt = mybir.dt
    F32, F16 = dt.float32, dt.float16

    nc = bacc.Bacc("TRN2", target_bir_lowering=False, debug=False)

    Mi = nc.dram_tensor("Mi", [PAIRS, 128, 2048], F16, kind="ExternalInput").ap()
    C = nc.dram_tensor("C", [128, 384], F16, kind="ExternalInput").ap()
    O = nc.dram_tensor("O", [PAIRS, 128, 1024], F16, kind="ExternalOutput").ap()

    with tile.TileContext(nc) as tc:
        with tc.tile_pool(name="const", bufs=1) as cp, \
             tc.tile_pool(name="sbuf", bufs=2) as sb, \
             tc.tile_pool(name="psW", bufs=3, space="PSUM") as psW, \
             tc.tile_pool(name="psZ", bufs=3, space="PSUM") as psZ, \
             tc.tile_pool(name="psT", bufs=2, space="PSUM") as psT:
            Ct = cp.tile([128, 384], F16, tag="C")
            nc.sync.dma_start(Ct[:], C)
            AT = Ct[:, 0:128]
            BT = Ct[:, 128:256]
            IdT = Ct[:, 256:384]

            st = {}

            def dma_in(pr):
                xin = sb.tile([128, 2048], F16, tag="xin", bufs=7, name="xin")
                nc.sync.dma_start(xin[:], Mi[pr])
                st[pr, "x"] = xin

            def presums(pr, c):
                # XP = [xs_i0 | Ps | xs_i1 | Pa]; U-moving = XP[:, 256:768]
                x5 = st[pr, "x"][:, c * 1024:(c + 1) * 1024].rearrange(
                    "p (b i j l) -> p b i j l", b=2, i=2, j=2)
                XP = sb.tile([128, 1024], F16, tag=f"XP{c}", bufs=4, name=f"XP{c}")
                xsv = XP[:].rearrange("p (i k b l) -> p i k b l", i=2, k=2, b=2)
                # xs (J-sum) into slots 0 and 2 via strided out
                nc.gpsimd.tensor_add(xsv[:, :, 0], x5[:, :, :, 0].rearrange(
                    "p b i l -> p i b l"), x5[:, :, :, 1].rearrange(
                    "p b i l -> p i b l"))
                # Ps = xs_i0 + xs_i1 -> slot 1
                nc.gpsimd.tensor_add(XP[:, 256:512], XP[:, 0:256], XP[:, 512:768])
                # Pa = x_i0j1 + x_i1j1 -> slot 3
                nc.gpsimd.tensor_add(XP[:, 768:1024].rearrange(
                    "p (b l) -> p b l", b=2), x5[:, :, 0, 1], x5[:, :, 1, 1])
                st[pr, c, "XP"] = XP

            def s1(pr, c):
                x5 = st[pr, "x"][:, c * 1024:(c + 1) * 1024].rearrange(
                    "p (b i j l) -> p b i j l", b=2, i=2, j=2)
                XP = st[pr, c, "XP"]
                U = psW.tile([128, 512], F32, tag="w", name=f"U{c}")
                V = psW.tile([128, 512], F32, tag="w", name=f"V{c}")
                mm = nc.tensor.matmul
                mm(U[:], AT, XP[:, 256:768], start=True, stop=True)
                mm(V[:, 0:256], AT, XP[:, 768:1024], start=True, stop=True)
                mm(V[:, 256:512], AT, x5[:, :, 1, 1], start=True, stop=True)
                st[pr, c, "UV"] = (U, V)

            def split_w(pr, c):
                U, V = st[pr, c, "UV"]
                PK = sb.tile([128, 2048], F16, tag=f"PK{c}", bufs=4, name=f"PK{c}")
                nc.scalar.copy(PK[:, 0:512], U[:])
                nc.scalar.copy(PK[:, 512:1024], V[:])
                nc.vector.tensor_sub(PK[:, 1024:1536], U[:], PK[:, 0:512])
                nc.vector.tensor_sub(PK[:, 1536:2048], V[:], PK[:, 512:1024])
                st[pr, c, "PK"] = PK

            def trans_w(pr, c):
                PK = st[pr, c, "PK"]
                PKT = sb.tile([128, 2048], F16, tag=f"PKT{c}", bufs=4,
                              name=f"PKT{c}")
                if c == 0:
                    nc.sync.dma_start_transpose(
                        PKT[:].rearrange("p (b f) -> p b f", b=16), PK[:])
                else:
                    for h in (0, 1):
                        TP = psT.tile([128, 1024], F16, tag="tp", name=f"TP{h}")
                        for b in range(8):
                            nc.tensor.matmul(
                                TP[:, b * 128:(b + 1) * 128],
                                PK[:, (h * 8 + b) * 128:(h * 8 + b + 1) * 128],
                                IdT, is_transpose=True,
                                start=(b == 0), stop=(b == 7))
                        if h == 0:
                            nc.scalar.copy(PKT[:, 0:1024], TP[:])
                        else:
                            nc.vector.tensor_copy(PKT[:, 1024:2048], TP[:])
                st[pr, c, "PKT"] = PKT

            def s2(pr, c):
                PKT = st[pr, c, "PKT"]
                za = psZ.tile([128, 512], F32, tag="z", name=f"za{c}")
                zb = psZ.tile([128, 512], F32, tag="z", name=f"zb{c}")
                nc.tensor.matmul(za[:], AT, PKT[:, 0:512], start=True, stop=False)
                nc.tensor.matmul(za[:], AT, PKT[:, 1024:1536],
                                 start=False, stop=True)
                nc.tensor.matmul(zb[:], AT, PKT[:, 512:1024],
                                 start=True, stop=False)
                nc.tensor.matmul(zb[:], AT, PKT[:, 1536:2048],
                                 start=False, stop=True)
                if c == 0:
                    z0s = sb.tile([128, 1024], F32, tag="z0s", bufs=4, name="z0s")
                    nc.scalar.mul(z0s[:, 0:512], za[:], OSCALE)
                    nc.vector.tensor_scalar_mul(z0s[:, 512:1024], zb[:], OSCALE)
                    st[pr, "z0s"] = z0s
                else:
                    t = sb.tile([128, 1024], F32, tag="t", bufs=4, name="t")
                    z0s = st[pr, "z0s"]
                    nc.vector.tensor_mul(t[:, 0:512], z0s[:, 0:512], za[:])
                    nc.vector.tensor_mul(t[:, 512:1024], z0s[:, 512:1024], zb[:])
                    st[pr, "t"] = t

            def qpres(pr):
                t = st[pr, "t"]
                t4 = t[:].rearrange("p (k f) -> p k f", k=4)
                Q4 = sb.tile([128, 768], F32, tag="Q4", bufs=4, name="Q4")
                q4 = Q4[:].rearrange("p (k f) -> p k f", k=3)
                nc.gpsimd.tensor_sub(q4[:, 2], t4[:, 1], t4[:, 3])   # D1
                nc.gpsimd.tensor_sub(q4[:, 0], t4[:, 0], t4[:, 2])   # Di0
                nc.gpsimd.tensor_sub(q4[:, 0], q4[:, 0], q4[:, 2])   # Dd
                nc.gpsimd.tensor_sub(q4[:, 1], t4[:, 2], t4[:, 3])   # Ed
                st[pr, "Q4"] = Q4

            def split_q(pr):
                # QP = [Ddh|Edh|D1h | Ddl|Edl|D1l] (1536); E1 split from t slice
                Q4 = st[pr, "Q4"]
                t = st[pr, "t"]
                QP = sb.tile([128, 1536], F16, tag="QP", bufs=4, name="QP")
                nc.scalar.copy(QP[:, 0:768], Q4[:])
                nc.vector.tensor_sub(QP[:, 768:1536], Q4[:], QP[:, 0:768])
                EP = sb.tile([128, 512], F16, tag="EP", bufs=4, name="EP")
                nc.scalar.copy(EP[:, 0:256], t[:, 768:1024])
                nc.vector.tensor_sub(EP[:, 256:512], t[:, 768:1024], EP[:, 0:256])
                st[pr, "QP"] = QP
                st[pr, "EP"] = EP

            def mob1(pr):
                QP = st[pr, "QP"]
                EP = st[pr, "EP"]
                Up = psW.tile([128, 512], F32, tag="w", name="Up")
                Vp = psW.tile([128, 512], F32, tag="w", name="Vp")
                mm = nc.tensor.matmul
                mm(Up[:], BT, QP[:, 0:512], start=True, stop=False)       # [Ddh|Edh]
                mm(Up[:], BT, QP[:, 768:1280], start=False, stop=True)    # [Ddl|Edl]
                mm(Vp[:, 0:256], BT, QP[:, 512:768], start=True, stop=False)   # D1h
                mm(Vp[:, 0:256], BT, QP[:, 1280:1536], start=False, stop=True) # D1l
                mm(Vp[:, 256:512], BT, EP[:, 0:256], start=True, stop=False)   # E1h
                mm(Vp[:, 256:512], BT, EP[:, 256:512], start=False, stop=True) # E1l
                st[pr, "UVp"] = (Up, Vp)

            def split_u(pr):
                Up, Vp = st[pr, "UVp"]
                MP = sb.tile([128, 2048], F16, tag="MP", bufs=4, name="MP")
                nc.scalar.copy(MP[:, 0:512], Up[:])
                nc.scalar.copy(MP[:, 512:1024], Vp[:])
                nc.vector.tensor_sub(MP[:, 1024:1536], Up[:], MP[:, 0:512])
                nc.vector.tensor_sub(MP[:, 1536:2048], Vp[:], MP[:, 512:1024])
                st[pr, "MP"] = MP

            def trans_u(pr):
                MP = st[pr, "MP"]
                MPT = sb.tile([128, 2048], F16, tag="MPT", bufs=4, name="MPT")
                # hi half on XBAR (scalar queue), lo half on PE
                nc.scalar.dma_start_transpose(
                    MPT[:, 0:1024].rearrange("p (b f) -> p b f", b=8),
                    MP[:, 0:1024])
                TU = psT.tile([128, 1024], F16, tag="tp", name="TU")
                for b in range(8):
                    nc.tensor.matmul(
                        TU[:, b * 128:(b + 1) * 128],
                        MP[:, (8 + b) * 128:(9 + b) * 128], IdT,
                        is_transpose=True, start=(b == 0), stop=(b == 7))
                nc.vector.tensor_copy(MPT[:, 1024:2048], TU[:])
                st[pr, "MPT"] = MPT

            def mob2(pr):
                MPT = st[pr, "MPT"]
                oa = psZ.tile([128, 512], F32, tag="z", name="oa")
                ob = psZ.tile([128, 512], F32, tag="z", name="ob")
                nc.tensor.matmul(oa[:], BT, MPT[:, 0:512], start=True, stop=False)
                nc.tensor.matmul(oa[:], BT, MPT[:, 1024:1536],
                                 start=False, stop=True)
                nc.tensor.matmul(ob[:], BT, MPT[:, 512:1024],
                                 start=True, stop=False)
                nc.tensor.matmul(ob[:], BT, MPT[:, 1536:2048],
                                 start=False, stop=True)
                st[pr, "o"] = (oa, ob)

            def outc(pr):
                oa, ob = st[pr, "o"]
                osb = sb.tile([128, 1024], F16, tag="osb", bufs=3, name="osb")
                nc.scalar.copy(osb[:, 0:512], oa[:])
                nc.scalar.copy(osb[:, 512:1024], ob[:])
                nc.sync.dma_start(O[pr], osb[:])

            G = GROUP
            ngroups = (PAIRS + G - 1) // G

            def grp(i):
                return range(i * G, min((i + 1) * G, PAIRS))

            for pr in grp(0):
                dma_in(pr)
            for g in range(ngroups):
                prs = grp(g)
                if g + 1 < ngroups:
                    for pr in grp(g + 1):
                        dma_in(pr)
                for c in (0, 1):
                    for pr in prs:
                        presums(pr, c)
                    for pr in prs:
                        s1(pr, c)
                    for pr in prs:
                        split_w(pr, c)
                    for pr in prs:
                        trans_w(pr, c)
                    for pr in prs:
                        s2(pr, c)
                for pr in prs:
                    qpres(pr)
                for pr in prs:
                    split_q(pr)
                for pr in prs:
                    mob1(pr)
                for pr in prs:
                    split_u(pr)
                for pr in prs:
                    trans_u(pr)
                for pr in prs:
                    mob2(pr)
                for pr in prs:
                    outc(pr)

    nc.compile()
    return nc


@functools.lru_cache(maxsize=1)
def _get_nc():
    return _build()


def _host_in(M):
    M16 = np.asarray(M, dtype=np.float32).astype(np.float16)
    M6 = M16.reshape(NCORES, PAIRS, 2, 2, 2, 128, 2, 128)
    Mi = np.ascontiguousarray(M6.transpose(0, 1, 5, 3, 2, 4, 6, 7))
    return Mi.reshape(NCORES, PAIRS, 128, 2048)


def _host_out(Os):
    O = np.stack(Os).reshape(NCORES, PAIRS, 128, 2, 2, 2, 128)
    out = np.ascontiguousarray(O.transpose(0, 1, 5, 3, 2, 4, 6))
    out = out.reshape(BATCH, L).astype(np.float32) * 65536.0
    return out[:, :, None, None]


def _run(M, trace=False):
    from concourse.bass_utils import run_bass_kernel_spmd
    nc = _get_nc()
    AT7, BT7 = _constants()
    Id = np.eye(128, dtype=np.float16)
    C = np.concatenate([AT7, BT7, Id], axis=1)
    Mi = _host_in(M)
    in_maps = [{"Mi": Mi[k], "C": C} for k in range(NCORES)]
    res = run_bass_kernel_spmd(nc, in_maps, list(range(NCORES)), trace=trace)
    out = _host_out([res.results[k]["O"] for k in range(NCORES)])
    return out, res


def kernel(M):
    try:
        out, _ = _run(M, trace=False)
    except Exception:
        out, _ = _run(M, trace=False)
    return out
